# revision 4
# baseline (speedup 1.0000x reference)
"""Trainium2 Bass kernel for BatchPPRFeatures:
    out[i] = sum_k ppr_scores[i,k] * x[ppr_idx[i,k]]   (N=100000, K=32, D=128)

Strategy (8 NeuronCores, node-parallel):
- Shard output rows across 8 cores (12500 rows/core, padded to 13312 = 104
  tiles of 128). x (converted to fp16) is replicated to every core.
- The gather runs via gpsimd dma_gather (SWDGE) with int16 indices. Since
  int16 limits a gather call to <32768 table rows, x is split into 4 chunks
  of 25000 rows; each output tile's 4096 (i,k) entries are bucketed by chunk
  on the host into fixed-capacity segments (CAP=1280 slots, padded with
  index 0 / score 0).
- Gathered slots land as [slot%128 -> partition, slot//128 -> block]. The
  weighted reduction is 40 accumulating PSUM matmuls per tile with one-hot
  "scatter" matrices W[p, m] = (iota==tc)*sp built on the VectorEngine from
  host-prepared per-slot target-row / score tensors.
- 4 SWDGE queues are used round-robin (one per chunk) - this parallelizes
  descriptor-ring drain and is ~3.5x faster than a single queue.
"""

import sys

sys.path.insert(0, "/opt/trn_rl_repo")

import numpy as np

N = 100000
D = 128
K = 32
N_CORES = 8
N_CHUNKS = 4
CHUNK = N // N_CHUNKS            # 25000 rows per chunk (int16-addressable)
ROWS_PER_CORE = N // N_CORES     # 12500
GROUP = 4                        # tiles per gather call group
CAP = 1280                       # slots per (tile, chunk) segment, mult of 128
BLOCKS_SEG = CAP // 128          # 10 blocks per segment
TILES = 104                      # ceil(12500/128) padded to GROUP multiple
GROUPS = TILES // GROUP          # 26
ROWS_PAD = TILES * 128           # 13312
BLOCKS_TILE = N_CHUNKS * BLOCKS_SEG          # 40 blocks per tile
NBLOCKS = TILES * BLOCKS_TILE                # 4160 block columns (tc/sp)
CALL_IDX = GROUP * CAP                       # 5120 indices per gather call
IDX_COLS = GROUPS * N_CHUNKS * (CALL_IDX // 16)  # 33280 int16 cols

_prog_cache = {}


def _build_program():
    """Build + compile the (input-independent) SPMD Bass program."""
    if "nc" in _prog_cache:
        return _prog_cache["nc"]
    from concourse import bacc, mybir, tile

    F16 = mybir.dt.float16
    F32 = mybir.dt.float32
    I16 = mybir.dt.int16

    nc = bacc.Bacc(
        "TRN2",
        target_bir_lowering=False,
        debug=False,
        num_devices=N_CORES,
        num_swdge_queues=4,
    )
    x_d = nc.dram_tensor("x", [N, D], F16, kind="ExternalInput")
    idx_d = nc.dram_tensor("idx16", [128, IDX_COLS], I16, kind="ExternalInput")
    tc_d = nc.dram_tensor("tcol", [128, NBLOCKS], F32, kind="ExternalInput")
    sp_d = nc.dram_tensor("sval", [128, NBLOCKS], F32, kind="ExternalInput")
    iota_d = nc.dram_tensor("iota", [128, 128], F16, kind="ExternalInput")
    out_d = nc.dram_tensor("out", [ROWS_PAD, D], F32, kind="ExternalOutput")

    with tile.TileContext(nc) as tc:
        with (
            tc.tile_pool(name="const", bufs=1) as cpool,
            tc.tile_pool(name="idxp", bufs=3) as idxp,
            tc.tile_pool(name="gp", bufs=2) as gpool,
            tc.tile_pool(name="wp", bufs=8) as wpool,
            tc.tile_pool(name="op", bufs=4) as opool,
            tc.tile_pool(name="ps", bufs=4, space="PSUM") as pspool,
        ):
            iota_sb = cpool.tile([128, 128], F16)
            nc.sync.dma_start(out=iota_sb[:], in_=iota_d[:])
            tc_sb = cpool.tile([128, NBLOCKS], F32)
            nc.sync.dma_start(out=tc_sb[:], in_=tc_d[:])
            sp_sb = cpool.tile([128, NBLOCKS], F32)
            nc.sync.dma_start(out=sp_sb[:], in_=sp_d[:])

            for g in range(GROUPS):
                idx_sb = idxp.tile([128, N_CHUNKS * CALL_IDX // 16], I16, tag="idx")
                nc.scalar.dma_start(
                    out=idx_sb[:],
                    in_=idx_d[
                        :,
                        g * N_CHUNKS * CALL_IDX // 16 : (g + 1)
                        * N_CHUNKS
                        * CALL_IDX
                        // 16,
                    ],
                )
                gs = []
                for c in range(N_CHUNKS):
                    g_sb = gpool.tile(
                        [128, GROUP * BLOCKS_SEG * D], F16, tag=f"g{c}"
                    )
                    nc.gpsimd.dma_gather(
                        out_ap=g_sb[:].rearrange("p (b d) -> p b d", d=D),
                        in_ap=x_d[c * CHUNK : (c + 1) * CHUNK, :],
                        idxs_ap=idx_sb[
                            :, c * CALL_IDX // 16 : (c + 1) * CALL_IDX // 16
                        ],
                        num_idxs=CALL_IDX,
                        num_idxs_reg=CALL_IDX,
                        elem_size=D,
                        single_packet=False,
                        queue_num=c,
                    )
                    gs.append(g_sb)

                for t in range(GROUP):
                    T = g * GROUP + t
                    ps = pspool.tile([128, D], F32, space="PSUM")
                    first = True
                    for c in range(N_CHUNKS):
                        for b in range(BLOCKS_SEG):
                            gb = (T * N_CHUNKS + c) * BLOCKS_SEG + b
                            w = wpool.tile([128, 128], F16, tag="w")
                            nc.vector.tensor_scalar(
                                out=w[:],
                                in0=iota_sb[:],
                                scalar1=tc_sb[:, gb : gb + 1],
                                scalar2=sp_sb[:, gb : gb + 1],
                                op0=mybir.AluOpType.is_equal,
                                op1=mybir.AluOpType.mult,
                            )
                            nc.tensor.matmul(
                                out=ps[:],
                                lhsT=w[:],
                                rhs=gs[c][
                                    :, (t * BLOCKS_SEG + b) * D : (t * BLOCKS_SEG + b + 1) * D
                                ],
                                start=first,
                                stop=(c == N_CHUNKS - 1 and b == BLOCKS_SEG - 1),
                            )
                            first = False
                    o_sb = opool.tile([128, D], F32, tag="o")
                    nc.scalar.tensor_copy(out=o_sb[:], in_=ps[:])
                    nc.sync.dma_start(
                        out=out_d[T * 128 : (T + 1) * 128, :], in_=o_sb[:]
                    )

    nc.compile()
    _prog_cache["nc"] = nc
    return nc


def _prep_core_inputs(idx_core, sc_core):
    """Bucket one core's (padded) indices by chunk into fixed-cap segments.

    idx_core: [ROWS_PAD, K] int32, sc_core: [ROWS_PAD, K] float32.
    Returns (idx16 [128, IDX_COLS] int16, tc [128, NBLOCKS] f32,
             sp [128, NBLOCKS] f32).
    """
    seg_idx = np.zeros((TILES, N_CHUNKS, CAP), dtype=np.int16)
    seg_tc = np.zeros((TILES, N_CHUNKS, CAP), dtype=np.float32)
    seg_sp = np.zeros((TILES, N_CHUNKS, CAP), dtype=np.float32)

    idx_t = idx_core.reshape(TILES, 128 * K)
    sc_t = sc_core.reshape(TILES, 128 * K)
    chunk_t = idx_t // CHUNK
    p_of_e = (np.arange(128 * K) // K).astype(np.float32)  # row within tile

    for T in range(TILES):
        ch = chunk_t[T]
        for c in range(N_CHUNKS):
            sel = np.nonzero(ch == c)[0]
            n = len(sel)
            if n > CAP:
                raise OverflowError(
                    f"segment overflow tile={T} chunk={c} n={n} > CAP={CAP}"
                )
            seg_idx[T, c, :n] = (idx_t[T, sel] - c * CHUNK).astype(np.int16)
            seg_tc[T, c, :n] = p_of_e[sel]
            seg_sp[T, c, :n] = sc_t[T, sel]

    # gather call lists: per (g, c) concat over t -> [5120]
    calls = (
        seg_idx.reshape(GROUPS, GROUP, N_CHUNKS, CAP)
        .transpose(0, 2, 1, 3)
        .reshape(GROUPS * N_CHUNKS, CALL_IDX)
    )
    # wrap into 16 partitions: entry j -> (j%16, j//16); replicate to 128
    wrapped = calls.reshape(GROUPS * N_CHUNKS, CALL_IDX // 16, 16).transpose(0, 2, 1)
    idx16 = np.tile(
        wrapped.transpose(1, 0, 2).reshape(16, IDX_COLS), (8, 1)
    ).astype(np.int16)

    # tc/sp: [128 partitions, NBLOCKS]; block col gb = (T*4 + c)*10 + b,
    # lane p of block gb = segment slot b*128 + p
    def to_blocks(a):
        # a: [TILES, N_CHUNKS, CAP] -> [128, NBLOCKS]
        return (
            a.reshape(TILES, N_CHUNKS, BLOCKS_SEG, 128)
            .transpose(3, 0, 1, 2)
            .reshape(128, NBLOCKS)
        )

    return (
        np.ascontiguousarray(idx16),
        np.ascontiguousarray(to_blocks(seg_tc)),
        np.ascontiguousarray(to_blocks(seg_sp)),
    )


def make_in_maps(x, ppr_idx, ppr_scores):
    x16 = np.asarray(x).astype(np.float16)
    ppr_idx = np.asarray(ppr_idx)
    ppr_scores = np.asarray(ppr_scores)
    iota = np.tile(np.arange(128, dtype=np.float16), (128, 1))

    idx_pad = np.zeros((N_CORES, ROWS_PAD, K), dtype=np.int64)
    sc_pad = np.zeros((N_CORES, ROWS_PAD, K), dtype=np.float32)
    # spread zero-weight padding rows' indices across chunks so no
    # per-(tile, chunk) segment overflows its fixed capacity
    idx_pad[:, ROWS_PER_CORE:] = (np.arange(K) % N_CHUNKS) * CHUNK
    idx_pad[:, :ROWS_PER_CORE] = ppr_idx.reshape(N_CORES, ROWS_PER_CORE, K)
    sc_pad[:, :ROWS_PER_CORE] = ppr_scores.reshape(N_CORES, ROWS_PER_CORE, K)

    in_maps = []
    for c in range(N_CORES):
        idx16, tcol, sval = _prep_core_inputs(idx_pad[c], sc_pad[c])
        in_maps.append(
            {"x": x16, "idx16": idx16, "tcol": tcol, "sval": sval, "iota": iota}
        )
    return in_maps


def kernel(x, ppr_idx, ppr_scores):
    from concourse.bass_utils import run_bass_kernel_spmd

    nc = _build_program()
    in_maps = make_in_maps(x, ppr_idx, ppr_scores)
    res = run_bass_kernel_spmd(nc, in_maps, core_ids=list(range(N_CORES)))
    out = np.concatenate(
        [res.results[c]["out"][:ROWS_PER_CORE] for c in range(N_CORES)], axis=0
    )
    return out.astype(np.float32)
